# revision 41
# baseline (speedup 1.0000x reference)
"""Distributed Trainium2 Bass kernel for a 3-layer GraphConv GNN.

Full inputs in, full output out. Internally: 8-way node-partitioned
message passing with halo exchange (AllGather of node features between
layers), per the standard graph-parallel sharding.

out = GC3(GC2(GC1(x)))  with  GC(h) = act(norm_dst * ((h*norm_src) @ W
aggregated over edges) + b)

Device pipeline per layer (per core):
  dma_gather (bf16 rows of the node table, per edge)
  -> one-hot tiles built on DVE (iota == dstw)
  -> accumulating TensorE matmuls into PSUM per 128-dst window (node-major)
  -> ACT eviction scaled by norm_dst*norm_src (per-partition)
  -> PE transpose -> TensorE transform (W, rank-1 bias) -> ACT relu
  -> AllGather the new node-feature slice for the next layer.
"""

import sys
import types

sys.path.insert(0, "/opt/trn_rl_repo")

import numpy as np
import ml_dtypes

bf16 = ml_dtypes.bfloat16
fp8 = ml_dtypes.float8_e4m3fn

# ---------------------------------------------------------------- constants
DIN = 128
HID = 128
DOUT = 64
NCORES = 8
WSZ = 128                    # dst nodes per window
NCH = 4                      # src chunks (core pairs)
BT = 8                       # one-hot tiles built per DVE instruction


def configure(n=100000, e=1600000, nwin=98, sb_sizes=None):
    """Set problem-size-derived module globals (small values for debugging)."""
    global N, E, NPC, NWIN, SLOT, CH_ROWS, TBL_ROWS, NT, SB_SIZES
    global SB_WINDOWS, CELL_TILE_OFF, SB_TILE_BASE, SB_K_TILES, SB_K_TILE_OFF
    global T_TOTAL, TOT_SLOTS
    N, E, NWIN = n, e, nwin
    NPC = N // NCORES
    SLOT = NWIN * WSZ
    CH_ROWS = 2 * SLOT
    assert CH_ROWS < 32768
    TBL_ROWS = NCORES * SLOT
    assert NPC <= SLOT

    base = max(1, int(round(E / NCORES / NWIN / NCH / 128)))
    NT = np.full((NWIN, NCH), base, np.int64)
    for _w in range(NWIN):
        NT[_w, _w % NCH] += 1
    NT[NWIN - 2:, :] = base + 1

    if sb_sizes is None:
        sb_sizes = [7] * 14
    assert sum(sb_sizes) == NWIN
    SB_SIZES = sb_sizes
    (SB_WINDOWS, CELL_TILE_OFF, SB_TILE_BASE, SB_K_TILES, SB_K_TILE_OFF,
     T_TOTAL) = _plan_layout()
    TOT_SLOTS = T_TOTAL * 128


def _plan_layout():
    """Static (data-independent) layout: slot offsets for every (sb, k, w) cell."""
    sb_windows = []
    w0 = 0
    for s in SB_SIZES:
        sb_windows.append(list(range(w0, w0 + s)))
        w0 += s

    # global tile/slot numbering: sb-major, then chunk, then window
    cell_tile_off = np.zeros((NWIN, NCH), np.int64)  # global tile idx of cell start
    sb_tile_base = []      # global tile idx of sb start
    sb_k_tiles = []        # [sb][k] -> tiles in that gather block
    sb_k_tile_off = []     # [sb][k] -> tile offset of block within sb
    t = 0
    for sbi, ws in enumerate(sb_windows):
        sb_tile_base.append(t)
        ktiles = []
        ktoff = []
        for k in range(NCH):
            ktoff.append(t - sb_tile_base[sbi])
            for w in ws:
                cell_tile_off[w, k] = t
                t += NT[w, k]
            ktiles.append(t - sb_tile_base[sbi] - ktoff[k])
        sb_k_tiles.append(ktiles)
        sb_k_tile_off.append(ktoff)
    total_tiles = t
    return (sb_windows, cell_tile_off, sb_tile_base, sb_k_tiles, sb_k_tile_off,
            total_tiles)


configure()


# ---------------------------------------------------------------- host prep
def _assign_cores(in_deg):
    """Serpentine assignment of nodes (sorted by in-degree desc) to 8 cores.

    Balances per-core edge counts to within a few edges; exactly NPC per core.
    """
    order = np.argsort(-in_deg, kind="stable")
    core_of = np.empty(N, np.int32)
    pattern = np.concatenate([np.arange(NCORES), np.arange(NCORES)[::-1]])
    core_of[order] = pattern[np.arange(N) % (2 * NCORES)]
    return core_of


def _pack_windows(node_ids, cdeg):
    """Greedy chunk-aware packing of one core's nodes into NWIN windows.

    cdeg: [n, NCH] per-chunk in-degree of each node. Returns (win, pos) arrays.
    Capacity per (w, k) = NT[w, k] * 128 edge slots; <= 128 nodes per window.
    """
    n = len(node_ids)
    rem = (NT * 128).astype(np.int64).copy()      # [NWIN, NCH] remaining edges
    cnt = np.zeros(NWIN, np.int64)                # nodes per window
    win = np.empty(n, np.int64)
    order = np.argsort(-cdeg.max(1), kind="stable")
    for i in order:
        v = cdeg[i]
        slack = rem - v                           # [NWIN, NCH]
        ms = slack.min(1)
        ms[cnt >= WSZ] = -(1 << 30)
        w = int(np.argmax(ms))                    # max worst-chunk slack
        win[i] = w
        rem[w] -= v
        cnt[w] += 1
    # repair: move nodes out of overflowing cells into windows with slack
    for _ in range(64):
        bad = np.argwhere(rem < 0)
        if len(bad) == 0:
            break
        w, k = bad[0]
        movers = [i for i in range(n) if win[i] == w and cdeg[i][k] > 0]
        movers.sort(key=lambda i: -cdeg[i][k])
        moved = False
        for i in movers:
            v = cdeg[i]
            ok = ((rem - v).min(1) >= 0) & (cnt < WSZ)
            ok[w] = False
            if ok.any():
                w2 = int(np.argmax(np.where(ok, (rem - v).min(1), -1)))
                win[i] = w2
                rem[w] += v
                rem[w2] -= v
                cnt[w] -= 1
                cnt[w2] += 1
                moved = True
                break
        if not moved:
            raise RuntimeError("window packing repair failed")
    assert (rem >= 0).all()
    # positions within window, in packing order
    pos = np.empty(n, np.int64)
    nxt = np.zeros(NWIN, np.int64)
    for i in order:
        pos[i] = nxt[win[i]]
        nxt[win[i]] += 1
    return win, pos


def prepare_host(x, src, dst, W1, b1, W2, b2, W3, b3):
    src = np.asarray(src).astype(np.int64)
    dst = np.asarray(dst).astype(np.int64)
    x = np.asarray(x, np.float32)

    out_deg = np.bincount(src, minlength=N).astype(np.float32)
    in_deg = np.bincount(dst, minlength=N).astype(np.float32)
    ns = 1.0 / np.sqrt(np.maximum(out_deg, 1.0))
    nd = 1.0 / np.sqrt(np.maximum(in_deg, 1.0))

    core_of = _assign_cores(in_deg)

    # chunk of a src node = its owning core pair
    chunk_of_src_node = core_of // 2

    # per-dst-node chunk-degree vectors
    cdeg = np.zeros((N, NCH), np.int64)
    np.add.at(cdeg, (dst, chunk_of_src_node[src]), 1)

    win_of = np.empty(N, np.int64)
    pos_of = np.empty(N, np.int64)
    for c in range(NCORES):
        ids = np.where(core_of == c)[0]
        w, p = _pack_windows(ids, cdeg[ids])
        win_of[ids] = w
        pos_of[ids] = p

    newrow = core_of.astype(np.int64) * SLOT + win_of * WSZ + pos_of

    # ---- edge arrays ----
    ec = core_of[dst]                       # owning core of each edge
    ew = win_of[dst]
    ek = chunk_of_src_node[src]             # src chunk
    eabs = newrow[src]                      # absolute table row of src
    eidx = eabs - ek * CH_ROWS              # gather idx within chunk
    edw = pos_of[dst]                       # dst partition within window

    sb_of_w = np.empty(NWIN, np.int64)
    for sbi, ws in enumerate(SB_WINDOWS):
        for w in ws:
            sb_of_w[w] = sbi

    # sort edges by (core, sb, chunk, window, idx)
    key = (((ec * 16 + sb_of_w[ew]) * NCH + ek) * NWIN + ew) * 40000 + eidx
    order = np.argsort(key, kind="stable")
    ec_s, ew_s, ek_s, eidx_s, edw_s, eabs_s = (
        a[order] for a in (ec, ew, ek, eidx, edw, eabs))

    # slot assignment: cell (c, w, k) occupies slots
    # [CELL_TILE_OFF[w,k]*128, +NT[w,k]*128); edges placed in sorted order.
    cell_id = (ec_s * NWIN + ew_s) * NCH + ek_s
    # rank within cell
    uniq, first = np.unique(cell_id, return_index=True)
    start_of = np.zeros(NCORES * NWIN * NCH, np.int64)
    start_of[uniq] = first
    rank = np.arange(len(cell_id)) - start_of[cell_id]
    cell_cap = np.tile(NT.reshape(-1) * 128, NCORES)
    assert (rank < cell_cap[cell_id]).all(), "cell overflow - packing failed"
    slot = CELL_TILE_OFF[ew_s, ek_s] * 128 + rank    # slot within core layout

    # per-core arrays
    idx_arrs, dstw_arrs, ohmap_arrs, sabs_arrs = [], [], [], []
    for c in range(NCORES):
        m = ec_s == c
        s_idx = np.zeros(TOT_SLOTS, np.int16)
        s_abs = np.zeros(TOT_SLOTS, np.int64)
        s_dw = np.full(TOT_SLOTS, -1.0, np.float32)
        sl = slot[m]
        s_idx[sl] = eidx_s[m].astype(np.int16)
        s_abs[sl] = eabs_s[m]
        s_dw[sl] = edw_s[m]
        sabs_arrs.append(s_abs)
        # pads: idx already 0 (valid row in chunk); dstw=-1 -> zero one-hot col

        # wrap idx per gather block (sb, k): local i -> [i%16, off/16 + i//16]
        idx16 = np.zeros((16, TOT_SLOTS // 16), np.int16)
        for sbi in range(len(SB_SIZES)):
            base = SB_TILE_BASE[sbi] * 128
            for k in range(NCH):
                off = base + SB_K_TILE_OFF[sbi][k] * 128
                nsl = SB_K_TILES[sbi][k] * 128
                blk = s_idx[off:off + nsl].reshape(-1, 16)      # [i//16, i%16]
                idx16[:, off // 16:(off + nsl) // 16] = blk.T
        idx_arrs.append(np.tile(idx16, (8, 1)))                 # replicate groups

        # host-built one-hot map, window-major tile order (sb, w, k, tile):
        # identical across layers, streamed by DMA instead of built on DVE.
        # fp8 (0.0/1.0 exact); slot partition p, tile t, dst column c.
        dw = np.ascontiguousarray(s_dw.reshape(T_TOTAL, 128).T)  # f32 cols
        perm = []
        for sbi, ws in enumerate(SB_WINDOWS):
            for w in ws:
                for k in range(NCH):
                    perm.extend(range(CELL_TILE_OFF[w, k],
                                      CELL_TILE_OFF[w, k] + NT[w, k]))
        dw_perm = np.ascontiguousarray(dw[:, np.array(perm)])    # [128, T_TOTAL]
        oh = (dw_perm[:, :, None] ==
              np.arange(128, dtype=np.float32)[None, None, :])
        ohmap_arrs.append(oh.astype(fp8))
        dstw_arrs.append(dw_perm.astype(bf16))

    # eviction scale / bias-row arrays per core
    ndns_arrs, nd_arrs, invnd_arrs = [], [], []
    for c in range(NCORES):
        sc = np.zeros((WSZ, NWIN), np.float32)
        sc3 = np.zeros((WSZ, NWIN), np.float32)
        invr = np.zeros(SLOT, np.float32)
        ids = np.where(core_of == c)[0]
        r = win_of[ids] * WSZ + pos_of[ids]
        sc[pos_of[ids], win_of[ids]] = nd[ids] * ns[ids]
        sc3[pos_of[ids], win_of[ids]] = nd[ids]
        invr[r] = 1.0 / nd[ids]
        ndns_arrs.append(sc)
        nd_arrs.append(sc3)
        invnd_arrs.append(invr[None, :].astype(bf16))

    # initial node table t0 = x * ns, permuted, bf16
    t0 = np.zeros((TBL_ROWS, DIN), np.float32)
    t0[newrow] = x * ns[:, None]
    t0 = t0.astype(bf16)

    # layer-1 message stream, host-expanded: the layer-1 gather reads the
    # static table t0, so bake the per-slot rows into a contiguous stream
    # ([128, T_TOTAL, DIN], partition-major) that the device streams with
    # plain (HWDGE) DMAs instead of per-edge gather descriptors.
    l1msgs_arrs = []
    for c in range(NCORES):
        l1 = t0[sabs_arrs[c]]                            # [TOT_SLOTS, DIN] bf16
        l1 = np.ascontiguousarray(
            l1.reshape(T_TOTAL, 128, DIN).transpose(1, 0, 2)
        ).reshape(128, T_TOTAL * DIN)
        l1msgs_arrs.append(l1)

    max_wtiles = int(NT.sum(1).max())
    iota_rep = np.tile(np.arange(128, dtype=np.float32), (128, max_wtiles)
                       ).astype(bf16)
    consts = dict(
        iota_rep=iota_rep,
        W1=np.asarray(W1, np.float32).astype(bf16),
        W2=np.asarray(W2, np.float32).astype(bf16),
        W3=np.asarray(W3, np.float32).astype(bf16),
        b1=np.asarray(b1, np.float32)[None, :].astype(bf16),
        b2=np.asarray(b2, np.float32)[None, :].astype(bf16),
        b3=np.asarray(b3, np.float32)[None, :].astype(bf16),
    )

    in_maps = []
    for c in range(NCORES):
        m = dict(consts)
        m.update(
            l1msgs=l1msgs_arrs[c], idx=idx_arrs[c], ohmap=ohmap_arrs[c],
            dstw=dstw_arrs[c],
            ndns=ndns_arrs[c], nd3=nd_arrs[c],
            invndrow=invnd_arrs[c],
        )
        in_maps.append(m)

    meta = dict(core_of=core_of, win_of=win_of, pos_of=pos_of)
    return in_maps, meta


# ---------------------------------------------------------------- device graph
def build_graph(has_bias=True):
    from concourse import bass, bacc, tile, mybir

    dt = mybir.dt
    nc = bacc.Bacc("TRN2", target_bir_lowering=False, debug=False,
                   num_devices=NCORES, num_swdge_queues=4)

    l1msgs_d = nc.dram_tensor("l1msgs", [128, T_TOTAL, DIN], dt.bfloat16,
                              kind="ExternalInput")
    idx_d = nc.dram_tensor("idx", [128, TOT_SLOTS // 16], dt.int16,
                           kind="ExternalInput")
    ohmap_d = nc.dram_tensor("ohmap", [128, T_TOTAL, 128], dt.float8e4,
                             kind="ExternalInput")
    dstw_d = nc.dram_tensor("dstw", [128, T_TOTAL], dt.bfloat16,
                            kind="ExternalInput")
    MAX_WTILES = int(NT.sum(1).max())
    iota_rep_d = nc.dram_tensor("iota_rep", [128, MAX_WTILES * 128], dt.bfloat16,
                                kind="ExternalInput")
    ndns_d = nc.dram_tensor("ndns", [WSZ, NWIN], dt.float32, kind="ExternalInput")
    nd3_d = nc.dram_tensor("nd3", [WSZ, NWIN], dt.float32, kind="ExternalInput")
    if has_bias:
        invnd_d = nc.dram_tensor("invndrow", [1, SLOT], dt.bfloat16,
                                 kind="ExternalInput")
    W_d = [nc.dram_tensor("W1", [DIN, HID], dt.bfloat16, kind="ExternalInput"),
           nc.dram_tensor("W2", [HID, HID], dt.bfloat16, kind="ExternalInput"),
           nc.dram_tensor("W3", [HID, DOUT], dt.bfloat16, kind="ExternalInput")]
    b_d = ([nc.dram_tensor("b1", [1, HID], dt.bfloat16, kind="ExternalInput"),
            nc.dram_tensor("b2", [1, HID], dt.bfloat16, kind="ExternalInput"),
            nc.dram_tensor("b3", [1, DOUT], dt.bfloat16, kind="ExternalInput")]
           if has_bias else [])
    out_d = nc.dram_tensor("out", [SLOT, DOUT], dt.float32, kind="ExternalOutput")

    MAX_SB_TILES = max(sum(SB_K_TILES[s]) for s in range(len(SB_SIZES)))

    with tile.TileContext(nc) as tc:
        with (
            tc.tile_pool(name="const", bufs=1) as constp,
            tc.tile_pool(name="msgs", bufs=2) as msgsp,
            tc.tile_pool(name="oh", bufs=4) as ohp,
            tc.tile_pool(name="ohsb", bufs=2) as ohsbp,
            tc.tile_pool(name="mp", bufs=4) as mpp,
            tc.tile_pool(name="tn", bufs=1) as tnp,
            tc.tile_pool(name="psA", bufs=4, space="PSUM") as psA,
            tc.tile_pool(name="psB", bufs=3, space="PSUM") as psB,
            tc.tile_pool(name="dram", bufs=1, space="DRAM") as dram,
        ):
            # constants
            ndns_t = constp.tile([WSZ, NWIN], dt.float32)
            nd3_t = constp.tile([WSZ, NWIN], dt.float32)
            invnd_t = constp.tile([1, SLOT], dt.bfloat16, name="invnd_t") if has_bias else None
            # whole-run idx / dstw tables, loaded once (layer-invariant)
            idx_all = constp.tile([128, TOT_SLOTS // 16], dt.int16, name="idx_all")
            dstw_all = constp.tile([128, T_TOTAL], dt.bfloat16, name="dstw_all")
            iota_rep_t = constp.tile([128, MAX_WTILES, 128], dt.bfloat16,
                                     name="iota_rep_t")
            nc.scalar.dma_start(idx_all[:], idx_d.ap()[:])
            nc.scalar.dma_start(dstw_all[:], dstw_d.ap()[:])
            nc.scalar.dma_start(
                iota_rep_t[:],
                iota_rep_d.ap()[:].rearrange("p (t c) -> p t c", c=128))
            nc.sync.dma_start(ndns_t[:], ndns_d.ap()[:])
            nc.sync.dma_start(nd3_t[:], nd3_d.ap()[:])
            if has_bias:
                nc.sync.dma_start(invnd_t[:], invnd_d.ap()[:])
            W_t, b_t = [], []
            for l in range(3):
                hout = DOUT if l == 2 else HID
                wt = constp.tile([DIN, hout], dt.bfloat16, name=f"w{l}")
                nc.sync.dma_start(wt[:], W_d[l].ap()[:])
                W_t.append(wt)
                if has_bias:
                    bt = constp.tile([1, hout], dt.bfloat16, name=f"bt{l}")
                    nc.sync.dma_start(bt[:], b_d[l].ap()[:])
                    b_t.append(bt)

            slice_dr = [dram.tile([SLOT, HID], dt.bfloat16, name=f"sl{l}")
                        for l in range(2)]
            full_dr = [dram.tile([TBL_ROWS, HID], dt.bfloat16,
                                 addr_space="Shared", name=f"fl{l}")
                       for l in range(2)]

            for layer in range(3):
                hout = DOUT if layer == 2 else HID
                table_ap = None if layer == 0 else full_dr[layer - 1]
                scale_t = nd3_t if layer == 2 else ndns_t
                bias_lhs = invnd_t

                if layer == 2:
                    out_sb = tnp.tile([128, NWIN, DOUT], dt.float32, name="outsb", tag="tnext")
                else:
                    tnext = tnp.tile([128, NWIN, HID], dt.bfloat16,
                                     name=f"tnext{layer}", tag="tnext")

                for sbi, ws in enumerate(SB_WINDOWS):
                    sb_base = SB_TILE_BASE[sbi]
                    ntiles_sb = sum(SB_K_TILES[sbi])

                    MAX_K_TILES = max(max(kt) for kt in SB_K_TILES)
                    msgs_k = [msgsp.tile([128, MAX_K_TILES, 128], dt.bfloat16,
                                         name=f"msgs{layer}_{sbi}_{k}",
                                         tag=f"msgs{k}")
                              for k in range(NCH)]
                    MAX_WTILES = int(NT.sum(1).max())

                    if layer == 0:
                        ohs = ohsbp.tile([128, MAX_SB_TILES, 128],
                                         dt.float8e4,
                                         name=f"ohs{layer}_{sbi}", tag="ohs")
                        nc.scalar.dma_start(
                            ohs[:, :ntiles_sb, :],
                            ohmap_d.ap()[:, sb_base:sb_base + ntiles_sb, :])

                    for k in range(NCH):
                        ktiles = SB_K_TILES[sbi][k]
                        koff = SB_K_TILE_OFF[sbi][k]
                        nsl = ktiles * 128
                        goff = (sb_base + koff) * 128
                        if layer == 0:
                            # host-expanded message stream: plain HWDGE DMA,
                            # alternating rings
                            eng = nc.sync if k % 2 == 0 else nc.scalar
                            eng.dma_start(
                                msgs_k[k][:, :ktiles, :],
                                l1msgs_d.ap()[:, sb_base + koff:
                                              sb_base + koff + ktiles, :])
                            continue
                        nc.gpsimd.dma_gather(
                            msgs_k[k][:, :ktiles, :],
                            table_ap[k * CH_ROWS:(k + 1) * CH_ROWS, :],
                            idx_all[:, goff // 16:(goff + nsl) // 16],
                            num_idxs=nsl, num_idxs_reg=nsl, elem_size=128,
                            single_packet=False, queue_num=k,
                        )

                    woff = 0
                    for w in ws:
                        accA = psA.tile([128, 128], dt.float32,
                                        name=f"accA{layer}_{w}", tag="psA")
                        ntot = int(NT[w].sum())
                        # whole window's one-hots in one DVE instruction
                        # (dstw is window-major; msgs stays chunk-major)
                        if layer == 0:
                            oh_tile, oh_off = ohs, woff
                        else:
                            oht = ohp.tile([128, MAX_WTILES, 128],
                                           dt.bfloat16,
                                           name=f"oh{layer}_{w}", tag="ohv")
                            nc.vector.tensor_tensor(
                                oht[:, :ntot, :],
                                dstw_all[:, sb_base + woff:
                                         sb_base + woff + ntot]
                                    .unsqueeze(2)
                                    .broadcast_to([128, ntot, 128]),
                                iota_rep_t[:, :ntot, :],
                                op=mybir.AluOpType.is_equal,
                            )
                            oh_tile, oh_off = oht, 0
                        done = 0
                        for k in range(NCH):
                            base = (CELL_TILE_OFF[w, k] - sb_base
                                    - SB_K_TILE_OFF[sbi][k])
                            for j in range(int(NT[w, k])):
                                # acc[feat, dst] += msgs^T @ onehot
                                nc.tensor.matmul(
                                    accA[:], msgs_k[k][:, base + j, :],
                                    oh_tile[:, oh_off + done, :],
                                    start=(done == 0), stop=(done == ntot - 1),
                                )
                                done += 1
                        woff += ntot
                        # evict aggregated [feat, dst] (f32 -> bf16), unscaled
                        macc = mpp.tile([128, 128], dt.bfloat16,
                                        name=f"mp{layer}_{w}", tag="mp")
                        nc.scalar.activation(macc[:], accA[:],
                                             mybir.ActivationFunctionType.Copy)
                        # transform (contraction over feat) + rank-1 bias
                        # (bias pre-scaled by 1/nd; dst-norm applied at the end)
                        accB = psB.tile([128, hout], dt.float32,
                                        name=f"accB{layer}_{w}", tag="psB")
                        if has_bias:
                            nc.tensor.matmul(accB[:], macc[:], W_t[layer][:],
                                             start=True, stop=False)
                            nc.tensor.matmul(
                                accB[:],
                                bias_lhs[:, w * WSZ:(w + 1) * WSZ],
                                b_t[layer][:],
                                start=False, stop=True)
                        else:
                            nc.tensor.matmul(accB[:], macc[:], W_t[layer][:],
                                             start=True, stop=True)
                        # epilogue with dst-norm scale
                        if layer == 2:
                            nc.scalar.activation(
                                out_sb[:, w, :], accB[:],
                                mybir.ActivationFunctionType.Copy,
                                scale=scale_t[:, w:w + 1])
                        else:
                            nc.scalar.activation(
                                tnext[:, w, :], accB[:],
                                mybir.ActivationFunctionType.Relu,
                                scale=scale_t[:, w:w + 1])

                    # evict this superblock's windows as soon as they finish
                    g0, g1 = ws[0], ws[-1] + 1
                    if layer < 2:
                        nc.sync.dma_start(
                            slice_dr[layer][g0 * WSZ:g1 * WSZ, :]
                                .rearrange("(w p) h -> p w h", p=WSZ),
                            tnext[:, g0:g1, :])
                    else:
                        nc.sync.dma_start(
                            out_d.ap()[g0 * WSZ:g1 * WSZ, :]
                                .rearrange("(w p) h -> p w h", p=WSZ),
                            out_sb[:, g0:g1, :])

                if layer < 2:
                    nc.gpsimd.collective_compute(
                        "AllGather", mybir.AluOpType.bypass,
                        replica_groups=[list(range(NCORES))],
                        ins=[slice_dr[layer].opt()],
                        outs=[full_dr[layer].opt()],
                    )

    nc.compile()
    return nc


_CACHE = {}


def kernel(x, src, dst, W1, b1, W2, b2, W3, b3):
    from concourse import bass_utils

    in_maps, meta = prepare_host(x, src, dst, W1, b1, W2, b2, W3, b3)
    has_bias = any(np.abs(np.asarray(b)).max() > 0 for b in (b1, b2, b3))
    if not has_bias:
        for m in in_maps:
            m.pop("invndrow")
            m.pop("b1"), m.pop("b2"), m.pop("b3")
    if _CACHE.get("has_bias") != has_bias or "nc" not in _CACHE:
        _CACHE["nc"] = build_graph(has_bias)
        _CACHE["has_bias"] = has_bias
    nc = _CACHE["nc"]

    res = bass_utils.run_bass_kernel_spmd(
        nc, in_maps, core_ids=list(range(NCORES)), **_CACHE.get("run_kwargs", {}))
    _CACHE["last_result"] = res

    core_of, win_of, pos_of = meta["core_of"], meta["win_of"], meta["pos_of"]
    out = np.empty((N, DOUT), np.float32)
    rows = win_of * WSZ + pos_of
    for c in range(NCORES):
        ids = np.where(core_of == c)[0]
        out[ids] = res.results[c]["out"][rows[ids]]
    return out

